# revision 21
# baseline (speedup 1.0000x reference)
"""Trainium2 Bass kernel for nn_Connect_Cls (GNN edge-pair classifier).

Math refactor: for pairs (i, j),
    h[e] = concat(x[i], x[j]) @ W1 + b1 = (x @ W1_top)[i] + (x @ W1_bot)[j] + b1
so we precompute per-node tables A = x @ W1[:512], B = x @ W1[512:] (sharded
over nodes, AllGathered), then each edge is a gather + add.  b1 cancels out of
the BatchNorm entirely (it shifts h and mu equally), so it is never used.

BN refold: with s = gamma*rsqrt(var+eps) > 0 and t = beta - mu*s,
    relu(s*h + t) @ W2 = relu(h + t/s) @ (s ⊙rows W2)
so pass 2 needs only a single fused (add, max 0) op per chunk, with the scale
folded into W2 once.

Per core (8 cores, data-parallel over the 131072 edge pairs):
  phase 1: compute a 1024-node shard of the combined [8192, 2048] bf16 AB
           table on the PE (host supplies x^T bf16, so no on-device
           transposes), AllGather the full table.
  pass 1:  dma_gather (transposed: features on partitions) A[i] rows directly
           into the h tile, gather B[j] rows, h += B in place on DVE,
           bn_stats per feature chunk; first N_CACHE tiles stay SBUF-resident,
           the rest spill to a DRAM scratch (bf16).
  stats:   bn_aggr -> per-core sum/sumsq, AllReduce, then cb = t/s and
           W2' = s ⊙rows W2.
  pass 2:  reload spilled h tiles, hr = max(h + cb, 0) in place (DVE 2x mode
           for 6 chunks, ACT for 2), out = hr @ W2' on PE (contraction over
           features = partitions), + b2 via ACT copy+bias, write transposed
           output [2, E_core].
"""

import numpy as np

import concourse.bacc as bacc
import concourse.bass as bass
import concourse.mybir as mybir
import concourse.tile as tile
from concourse.bass_utils import run_bass_kernel_spmd
from concourse.library_config import mlp

f32 = mybir.dt.float32
bf16 = mybir.dt.bfloat16
i16 = mybir.dt.int16
OP = mybir.AluOpType
AF = mybir.ActivationFunctionType

N_NODES = 8192
F_IN = 512
F_MID = 1024
NCLS = 2
E = 65536
NCORES = 8
E_CORE = 2 * E // NCORES       # 16384 edges per core
NODES_CORE = N_NODES // NCORES  # 1024 nodes per core in phase 1
FC = F_MID // 128               # 8 feature chunks of 128
KC_IN = F_IN // 128             # 4 input-feature chunks
GE = 512                        # edges per gather tile
NT = E_CORE // GE               # 32 tiles
N_CACHE = 19                    # h tiles kept SBUF-resident (skip DRAM scratch)
N_ACT = 2                       # pass-2 relu chunks done on ACT (rest on DVE)
BN_EPS = 1e-5


ABLATE = set()  # timing experiments: {"bnstats", "gathers", "spill", "coll"}


class _StopBuild(Exception):
    pass


def build_program(for_timeline=False):
    """for_timeline=True builds a single-core, collective-free variant whose
    per-core instruction stream is identical except collectives become local
    DMA copies — used with TimelineSim for cost-model profiling."""
    ndev = 1 if for_timeline else NCORES
    nc = bacc.Bacc("TRN2", target_bir_lowering=False, debug=False,
                   num_devices=ndev)

    inpT = nc.dram_tensor("inpT_shard", [F_IN, NODES_CORE], bf16, kind="ExternalInput")
    w1 = nc.dram_tensor("w1", [2 * F_IN, F_MID], bf16, kind="ExternalInput")
    w2 = nc.dram_tensor("w2", [F_MID, NCLS], f32, kind="ExternalInput")
    gamma = nc.dram_tensor("gamma", [F_MID], f32, kind="ExternalInput")
    beta = nc.dram_tensor("beta", [F_MID], f32, kind="ExternalInput")
    b2 = nc.dram_tensor("b2", [NCLS], f32, kind="ExternalInput")
    idx_src = nc.dram_tensor("idx_src", [128, E_CORE // 16], i16, kind="ExternalInput")
    idx_dst = nc.dram_tensor("idx_dst", [128, E_CORE // 16], i16, kind="ExternalInput")
    outT = nc.dram_tensor("outT", [NCLS, E_CORE], f32, kind="ExternalOutput")

    groups = [list(range(NCORES))]

    with tile.TileContext(nc) as tc:
        with (
            tc.tile_pool(name="const", bufs=1) as cs,
            tc.tile_pool(name="sb", bufs=1) as sb,
            tc.tile_pool(name="psum", bufs=2, space="PSUM") as pp,
            tc.tile_pool(name="dram", bufs=1, space="DRAM") as dram,
        ):
            try:
                nc.gpsimd.load_library(mlp)

                # persistent small tiles (allocated below any phase-1 temps)
                idxs = cs.tile([128, 2, E_CORE // 16], i16)
                stats = cs.tile([128, FC, NT, 6], f32)
                w2_sb = cs.tile([128, FC, NCLS], f32)
                w2p = cs.tile([128, FC, NCLS], bf16)
                gam = cs.tile([128, FC], f32)
                bet = cs.tile([128, FC], f32)
                rgam = cs.tile([128, FC], f32)
                b2_sb = cs.tile([NCLS, 1], f32)
                eps_t = cs.tile([128, 1], f32)
                musig = cs.tile([128, 2 * FC], f32)
                musq = cs.tile([128, FC], f32)
                std = cs.tile([128, FC], f32)
                rstd = cs.tile([128, FC], f32)
                scale = cs.tile([128, FC], f32)
                inv_s = cs.tile([128, FC], f32)
                cb = cs.tile([128, FC], f32)
                mv = cs.tile([128, FC, 2], f32)
                ar_sb = cs.tile([128, 2 * FC], f32)
                msq = cs.tile([128, FC], f32)
                gsum = cs.tile([128, 2 * FC], f32)

                # ---------------- phase 1: node tables ----------------
                # load order: inT + W1 first (phase-1 critical path), then
                # everything pass-1/2 needs.
                ab_shard = dram.tile([NODES_CORE, 2 * F_MID], bf16)
                with (
                    tc.tile_pool(name="ph1", bufs=1) as p1,
                    tc.tile_pool(name="psum1", bufs=1, space="PSUM") as pp1,
                ):
                    # inT[:, kk, n] = x[n, kk*128 + p]; host supplies x^T
                    inT = p1.tile([128, KC_IN, NODES_CORE], bf16)
                    nc.sync.dma_start(
                        out=inT[:],
                        in_=inpT[:].rearrange("(k p) n -> p k n", p=128))
                    w1_sb = p1.tile([128, 2 * KC_IN, F_MID], bf16)  # W1 rows chunked
                    for kc in range(2 * KC_IN):
                        nc.sync.dma_start(out=w1_sb[:, kc, :],
                                          in_=w1[kc * 128:(kc + 1) * 128, :])

                    nc.sync.dma_start(out=idxs[:, 0, :], in_=idx_src[:])
                    nc.sync.dma_start(out=idxs[:, 1, :], in_=idx_dst[:])
                    nc.sync.dma_start(out=w2_sb[:],
                                      in_=w2[:].rearrange("(c p) n -> p c n", p=128))
                    nc.sync.dma_start(out=gam[:],
                                      in_=gamma[:].rearrange("(c p) -> p c", p=128))
                    nc.sync.dma_start(out=bet[:],
                                      in_=beta[:].rearrange("(c p) -> p c", p=128))
                    nc.sync.dma_start(out=b2_sb[:], in_=b2[:, None])
                    nc.gpsimd.memset(eps_t[:], BN_EPS)
                    nc.vector.reciprocal(out=rgam[:], in_=gam[:])

                    for t in range(NODES_CORE // 128):
                        for half in range(2):           # A then B
                            for ofc in range(2):        # 512-wide output chunks
                                mmps = pp1.tile([128, 512], f32, tag="mmps", bufs=3)
                                for kk in range(KC_IN):
                                    nc.tensor.matmul(
                                        out=mmps[:],
                                        lhsT=inT[:, kk, t * 128:(t + 1) * 128],
                                        rhs=w1_sb[:, half * KC_IN + kk,
                                                  ofc * 512:(ofc + 1) * 512],
                                        start=(kk == 0), stop=(kk == KC_IN - 1),
                                    )
                                absb = p1.tile([128, 512], bf16, tag="absb", bufs=3)
                                ceng = nc.vector if (t * 4 + half * 2 + ofc) % 2 else nc.scalar
                                if ceng is nc.scalar:
                                    ceng.activation(out=absb[:], in_=mmps[:],
                                                    func=AF.Identity)
                                else:
                                    ceng.tensor_copy(out=absb[:], in_=mmps[:])
                                nc.sync.dma_start(
                                    out=ab_shard[t * 128:(t + 1) * 128,
                                                 half * F_MID + ofc * 512:
                                                 half * F_MID + (ofc + 1) * 512],
                                    in_=absb[:])

                ab_full = dram.tile([N_NODES, 2 * F_MID], bf16,
                                    addr_space="Local" if for_timeline else "Shared")
                if for_timeline:
                    if "coll" not in ABLATE:
                        nc.sync.dma_start(out=ab_full[0:NODES_CORE, :], in_=ab_shard[:])
                else:
                    nc.gpsimd.collective_compute(
                        "AllGather", OP.bypass, replica_groups=groups,
                        ins=[ab_shard.opt()], outs=[ab_full.opt()])

                # ---------------- pass 1: gather + h + stats ----------------
                do_pass1 = "stop1" not in ABLATE
                do_stats = do_pass1 and "stop2" not in ABLATE
                do_pass2 = do_stats and "stop3" not in ABLATE

                h_scr = dram.tile([NT - N_CACHE, 128, FC, GE], bf16)

                # tile visit order: interleave spilled tiles among cached ones
                # so their extra spill/reload DMA hides behind cached-tile
                # compute in both passes.
                n_spill = NT - N_CACHE
                seq = []          # (tile_id, cache_slot or None, spill_slot or None)
                ci = si = 0
                for g in range(NT):
                    if si < n_spill and (g % 2 == 1 or ci >= N_CACHE):
                        seq.append((g, None, si)); si += 1
                    else:
                        seq.append((g, ci, None)); ci += 1

                N_POOL_ADD = 3    # h+=B chunks done on Pool (rest on DVE)

                with tc.tile_pool(name="hc", bufs=1) as hcp:
                    hcache = hcp.tile([128, N_CACHE, FC, GE], bf16)
                    haps = {}

                    def p1_gather(k):
                        g, cslot, _ = seq[k]
                        if cslot is not None:
                            hap = hcache[:, cslot, :, :]
                        else:
                            hh1 = sb.tile([128, FC, GE], bf16, tag="h", bufs=2)
                            hap = hh1[:]
                        haps[k] = hap
                        bgt = sb.tile([128, FC, GE], bf16, tag="bg", bufs=2)
                        isl = slice(g * (GE // 16), (g + 1) * (GE // 16))
                        if "gathers" not in ABLATE:
                            nc.gpsimd.dma_gather(
                                hap, ab_full[:, 0:F_MID], idxs[:, 0, isl],
                                GE, GE, F_MID, elem_step=2 * F_MID, transpose=True)
                            nc.gpsimd.dma_gather(
                                bgt[:], ab_full[:, F_MID:2 * F_MID],
                                idxs[:, 1, isl],
                                GE, GE, F_MID, elem_step=2 * F_MID, transpose=True)
                        return bgt

                    def p1_compute(k, bgt):
                        g, _, sslot = seq[k]
                        hap = haps[k]
                        for c in range(FC):
                            eng = nc.gpsimd if c >= FC - N_POOL_ADD else nc.vector
                            eng.tensor_tensor(out=hap[:, c, :], in0=hap[:, c, :],
                                              in1=bgt[:, c, :], op=OP.add)
                        if "bnstats" not in ABLATE:
                            for c in range(FC):
                                nc.vector.bn_stats(out=stats[:, c, g, :],
                                                   in_=hap[:, c, :])
                        if sslot is not None and "spill" not in ABLATE:
                            nc.sync.dma_start(out=h_scr[sslot], in_=hap)

                    if do_pass1:
                        prev_bg = p1_gather(0)
                        for k in range(NT):
                            nxt_bg = p1_gather(k + 1) if k + 1 < NT else None
                            p1_compute(k, prev_bg)
                            prev_bg = nxt_bg

                    # ---------------- stats: aggregate + AllReduce ----------------
                    if not do_stats:
                        raise _StopBuild
                    for c in range(FC):
                        nc.vector.bn_aggr(out=mv[:, c, :], in_=stats[:, c, :, :])
                    nc.vector.tensor_scalar_mul(out=ar_sb[:, 0:FC], in0=mv[:, :, 0],
                                                scalar1=float(E_CORE))
                    nc.vector.tensor_tensor(out=msq[:], in0=mv[:, :, 0],
                                            in1=mv[:, :, 0], op=OP.mult)
                    nc.vector.tensor_tensor(out=msq[:], in0=msq[:], in1=mv[:, :, 1],
                                            op=OP.add)
                    nc.vector.tensor_scalar_mul(out=ar_sb[:, FC:2 * FC], in0=msq[:],
                                                scalar1=float(E_CORE))
                    ar_in = dram.tile([128, 2 * FC], f32)
                    ar_out = dram.tile([128, 2 * FC], f32,
                                       addr_space="Local" if for_timeline else "Shared")
                    nc.sync.dma_start(out=ar_in[:], in_=ar_sb[:])
                    if for_timeline:
                        if "coll" not in ABLATE:
                            nc.sync.dma_start(out=ar_out[:], in_=ar_in[:])
                    else:
                        nc.gpsimd.collective_compute(
                            "AllReduce", OP.add, replica_groups=groups,
                            ins=[ar_in.opt()], outs=[ar_out.opt()])
                    if for_timeline and "coll" in ABLATE:
                        nc.sync.dma_start(out=gsum[:], in_=ar_in[:])
                    else:
                        nc.sync.dma_start(out=gsum[:], in_=ar_out[:])

                    # mu = gsum[0:FC]/2E, E[h^2] = gsum[FC:]/2E (one op)
                    inv_n = 1.0 / (2.0 * E)
                    nc.vector.tensor_scalar_mul(out=musig[:], in0=gsum[:],
                                                scalar1=inv_n)
                    mu = musig[:, 0:FC]
                    var = musig[:, FC:2 * FC]
                    nc.vector.tensor_tensor(out=musq[:], in0=mu, in1=mu,
                                            op=OP.mult)
                    nc.vector.tensor_tensor(out=var, in0=var, in1=musq[:],
                                            op=OP.subtract)
                    nc.scalar.activation(out=std[:], in_=var, func=AF.Sqrt,
                                         bias=eps_t[:, 0:1])
                    nc.vector.reciprocal(out=rstd[:], in_=std[:])

                    # refold (scale = gamma*rstd > 0 since gamma > 0):
                    #   W2' = scale ⊙rows W2;  cb = shift/scale = beta/scale - mu
                    nc.vector.tensor_tensor(out=scale[:], in0=gam[:], in1=rstd[:],
                                            op=OP.mult)
                    nc.vector.tensor_tensor(out=inv_s[:], in0=std[:], in1=rgam[:],
                                            op=OP.mult)
                    nc.vector.tensor_tensor(out=cb[:], in0=bet[:], in1=inv_s[:],
                                            op=OP.mult)
                    nc.vector.tensor_tensor(out=cb[:], in0=cb[:], in1=mu,
                                            op=OP.subtract)
                    for n in range(NCLS):
                        nc.vector.tensor_tensor(out=w2p[:, :, n], in0=w2_sb[:, :, n],
                                                in1=scale[:], op=OP.mult)

                    # ---------------- pass 2: relu(h+cb) @ W2' ----------------
                    if not do_pass2:
                        raise _StopBuild
                    def emit_out(g, ops):
                        # psum -> sbuf (+b2) on ACT, deferred 2 tiles so the
                        # in-order ACT queue never waits on PE completion.
                        ob = sb.tile([NCLS, GE], f32, tag="ob", bufs=4)
                        nc.scalar.activation(out=ob[:], in_=ops[:],
                                             func=AF.Identity,
                                             bias=b2_sb[:, 0:1], scale=1.0)
                        nc.sync.dma_start(out=outT[:, g * GE:(g + 1) * GE],
                                          in_=ob[:])

                    spilled_ks = [k for k, (_, _, s) in enumerate(seq)
                                  if s is not None]
                    hh_bufs = {}

                    def p2_reload(k):
                        _, _, sslot = seq[k]
                        hh = sb.tile([128, FC, GE], bf16, tag="h", bufs=2)
                        nc.sync.dma_start(out=hh[:], in_=h_scr[sslot])
                        hh_bufs[k] = hh

                    for j in range(min(2, len(spilled_ks))):
                        p2_reload(spilled_ks[j])
                    next_rl = 2

                    def p2_relu(k):
                        # max(h + cb, 0) in place: 6 chunks on DVE (2x mode),
                        # 2 on Pool. Runs RELU_AHEAD tiles ahead of the
                        # matmuls so PE waits are pre-satisfied and it never
                        # drops out of full clock.
                        nonlocal next_rl
                        g, cslot, sslot = seq[k]
                        if cslot is not None:
                            hhap = hcache[:, cslot, :, :]
                        else:
                            hhap = hh_bufs.pop(k)[:]
                            if next_rl < len(spilled_ks):
                                p2_reload(spilled_ks[next_rl])
                                next_rl += 1
                        for c in range(FC):
                            if c == FC - 1:
                                nc.scalar.activation(out=hhap[:, c, :],
                                                     in_=hhap[:, c, :],
                                                     func=AF.Relu,
                                                     bias=cb[:, c:c + 1],
                                                     scale=1.0)
                                continue
                            eng = nc.gpsimd if c == FC - 2 else nc.vector
                            eng.tensor_scalar(
                                out=hhap[:, c, :], in0=hhap[:, c, :],
                                scalar1=cb[:, c:c + 1], scalar2=0.0,
                                op0=OP.add, op1=OP.max)
                        return hhap

                    RELU_AHEAD = 2
                    relu_done = {}
                    for k in range(min(RELU_AHEAD, NT)):
                        relu_done[k] = p2_relu(k)
                    pending = []
                    for k in range(NT):
                        if k + RELU_AHEAD < NT:
                            relu_done[k + RELU_AHEAD] = p2_relu(k + RELU_AHEAD)
                        g = seq[k][0]
                        hhap = relu_done.pop(k)
                        ops = pp.tile([NCLS, GE], f32, tag="ops", bufs=5)
                        for c in range(FC):
                            nc.tensor.matmul(out=ops[:], lhsT=w2p[:, c, :],
                                             rhs=hhap[:, c, :],
                                             start=(c == 0), stop=(c == FC - 1))
                        pending.append((g, ops))
                        if len(pending) > 2:
                            emit_out(*pending.pop(0))
                    for gg, oo in pending:
                        emit_out(gg, oo)

            except _StopBuild:
                pass
    nc.compile()
    return nc


_NC = None


def _get_program():
    global _NC
    if _NC is None:
        _NC = build_program()
    return _NC


def _wrap_idx(col):
    """[E_CORE] int -> [128, E_CORE//16] int16 in dma_gather's wrapped layout."""
    w = col.astype(np.int16).reshape(-1, 16).T          # [16, E_CORE//16]
    return np.ascontiguousarray(np.tile(w, (8, 1)))     # replicate to 128 parts


def _to_bf16_bytes(a):
    """f32 ndarray -> bf16 (round-to-nearest-even) as uint16 view ndarray."""
    import jax.numpy as jnp
    return np.asarray(jnp.asarray(a, dtype=jnp.bfloat16))


def make_in_maps(input, conn_idx, disconn_idx, W1, gamma, beta, W2, b2):
    input = np.asarray(input, dtype=np.float32)
    W1 = np.asarray(W1, dtype=np.float32)
    W2 = np.ascontiguousarray(np.asarray(W2, dtype=np.float32))
    gamma = np.ascontiguousarray(np.asarray(gamma, dtype=np.float32))
    beta = np.ascontiguousarray(np.asarray(beta, dtype=np.float32))
    b2 = np.ascontiguousarray(np.asarray(b2, dtype=np.float32))
    conn_idx = np.asarray(conn_idx)
    disconn_idx = np.asarray(disconn_idx)

    w1_bf = _to_bf16_bytes(W1)
    inT_bf = _to_bf16_bytes(input.T)                    # [F_IN, N]

    in_maps = []
    ec2 = E_CORE // 2  # edges per core from each of conn/disconn
    for c in range(NCORES):
        pc = np.concatenate(
            [conn_idx[c * ec2:(c + 1) * ec2], disconn_idx[c * ec2:(c + 1) * ec2]],
            axis=0)  # [E_CORE, 2]
        in_maps.append({
            "inpT_shard": np.ascontiguousarray(
                inT_bf[:, c * NODES_CORE:(c + 1) * NODES_CORE]),
            "w1": w1_bf, "w2": W2, "gamma": gamma, "beta": beta, "b2": b2,
            "idx_src": _wrap_idx(pc[:, 0]),
            "idx_dst": _wrap_idx(pc[:, 1]),
        })
    return in_maps


def assemble_output(results):
    out = np.empty((2 * E, NCLS), dtype=np.float32)
    ec2 = E_CORE // 2
    for c in range(NCORES):
        r = results[c]["outT"]  # [NCLS, E_CORE]
        out[c * ec2:(c + 1) * ec2] = r[:, 0:ec2].T
        out[E + c * ec2:E + (c + 1) * ec2] = r[:, ec2:].T
    return out


def run(inputs, trace=False):
    nc = _get_program()
    in_maps = make_in_maps(
        inputs["input"], inputs["conn_idx"], inputs["disconn_idx"],
        inputs["W1"], inputs["gamma"], inputs["beta"], inputs["W2"],
        inputs["b2"])
    res = run_bass_kernel_spmd(nc, in_maps, list(range(NCORES)), trace=trace)
    return assemble_output(res.results), res


def kernel(**inputs):
    out, _ = run(inputs, trace=False)
    return out


# revision 30
# speedup vs baseline: 1.0285x; 1.0285x over previous
"""Trainium2 Bass kernel for nn_Connect_Cls (GNN edge-pair classifier).

Math refactor: for pairs (i, j),
    h[e] = concat(x[i], x[j]) @ W1 + b1 = (x @ W1_top)[i] + (x @ W1_bot)[j] + b1
so we precompute per-node tables A = x @ W1[:512], B = x @ W1[512:] (sharded
over nodes, AllGathered), then each edge is a gather + add.  b1 cancels out of
the BatchNorm entirely (it shifts h and mu equally), so it is never used.

BN refold: with s = gamma*rsqrt(var+eps) > 0 and t = beta - mu*s,
    relu(s*h + t) @ W2 = relu(h + t/s) @ (s ⊙rows W2)
so pass 2 needs only a single fused (add, max 0) op per chunk, with the scale
folded into W2 once.

Per core (8 cores, data-parallel over the 131072 edge pairs):
  phase 1: compute a 1024-node shard of the combined [8192, 2048] bf16 AB
           table on the PE (host supplies x^T bf16, so no on-device
           transposes), AllGather the full table.
  pass 1:  dma_gather (transposed: features on partitions) A[i] rows directly
           into the h tile, gather B[j] rows, h += B in place on DVE,
           bn_stats per feature chunk; first N_CACHE tiles stay SBUF-resident,
           the rest spill to a DRAM scratch (bf16).
  stats:   bn_aggr -> per-core sum/sumsq, AllReduce, then cb = t/s and
           W2' = s ⊙rows W2.
  pass 2:  reload spilled h tiles, hr = max(h + cb, 0) in place (DVE 2x mode
           for 6 chunks, ACT for 2), out = hr @ W2' on PE (contraction over
           features = partitions), + b2 via ACT copy+bias, write transposed
           output [2, E_core].
"""

import numpy as np

import concourse.bacc as bacc
import concourse.bass as bass
import concourse.mybir as mybir
import concourse.tile as tile
from concourse.bass_utils import run_bass_kernel_spmd
from concourse.library_config import mlp

f32 = mybir.dt.float32
bf16 = mybir.dt.bfloat16
fp8 = mybir.dt.float8e4
i16 = mybir.dt.int16
OP = mybir.AluOpType
AF = mybir.ActivationFunctionType

N_NODES = 8192
F_IN = 512
F_MID = 1024
NCLS = 2
E = 65536
NCORES = 8
E_CORE = 2 * E // NCORES       # 16384 edges per core
NODES_CORE = N_NODES // NCORES  # 1024 nodes per core in phase 1
FC = F_MID // 128               # 8 feature chunks of 128
KC_IN = F_IN // 128             # 4 input-feature chunks
GE = 512                        # edges per gather tile
NT = E_CORE // GE               # 32 tiles
N_CACHE = 19                    # h tiles kept SBUF-resident (skip DRAM scratch)
N_ACT = 2                       # pass-2 relu chunks done on ACT (rest on DVE)
BN_EPS = 1e-5


ABLATE = set()  # timing experiments: {"bnstats", "gathers", "spill", "coll"}


class _StopBuild(Exception):
    pass


def build_program(for_timeline=False):
    """for_timeline=True builds a single-core, collective-free variant whose
    per-core instruction stream is identical except collectives become local
    DMA copies — used with TimelineSim for cost-model profiling."""
    ndev = 1 if for_timeline else NCORES
    nc = bacc.Bacc("TRN2", target_bir_lowering=False, debug=False,
                   num_devices=ndev)

    inpT = nc.dram_tensor("inpT_shard", [F_IN, NODES_CORE], bf16, kind="ExternalInput")
    w1 = nc.dram_tensor("w1", [2 * F_IN, F_MID], bf16, kind="ExternalInput")
    w2 = nc.dram_tensor("w2", [F_MID, NCLS], f32, kind="ExternalInput")
    gamma = nc.dram_tensor("gamma", [F_MID], f32, kind="ExternalInput")
    beta = nc.dram_tensor("beta", [F_MID], f32, kind="ExternalInput")
    b2 = nc.dram_tensor("b2", [NCLS], f32, kind="ExternalInput")
    idx_src = nc.dram_tensor("idx_src", [128, E_CORE // 16], i16, kind="ExternalInput")
    idx_dst = nc.dram_tensor("idx_dst", [128, E_CORE // 16], i16, kind="ExternalInput")
    outT = nc.dram_tensor("outT", [NCLS, E_CORE], f32, kind="ExternalOutput")

    groups = [list(range(NCORES))]

    with tile.TileContext(nc) as tc:
        with (
            tc.tile_pool(name="const", bufs=1) as cs,
            tc.tile_pool(name="sb", bufs=1) as sb,
            tc.tile_pool(name="psum", bufs=2, space="PSUM") as pp,
            tc.tile_pool(name="dram", bufs=1, space="DRAM") as dram,
        ):
            try:
                nc.gpsimd.load_library(mlp)

                # persistent small tiles (allocated below any phase-1 temps)
                idxs = cs.tile([128, 2, E_CORE // 16], i16)
                stats = cs.tile([128, FC, NT, 6], f32)
                w2_sb = cs.tile([128, FC, NCLS], f32)
                w2p = cs.tile([128, FC, NCLS], bf16)
                gam = cs.tile([128, FC], f32)
                bet = cs.tile([128, FC], f32)
                rgam = cs.tile([128, FC], f32)
                b2_sb = cs.tile([NCLS, 1], f32)
                eps_t = cs.tile([128, 1], f32)
                musig = cs.tile([128, 2 * FC], f32)
                musq = cs.tile([128, FC], f32)
                std = cs.tile([128, FC], f32)
                rstd = cs.tile([128, FC], f32)
                scale = cs.tile([128, FC], f32)
                inv_s = cs.tile([128, FC], f32)
                cb = cs.tile([128, FC], f32)
                mv = cs.tile([128, FC, 2], f32)
                ar_sb = cs.tile([128, 2 * FC], f32)
                msq = cs.tile([128, FC], f32)
                gsum = cs.tile([128, 2 * FC], f32)

                # ---------------- phase 1: node tables ----------------
                # load order: inT + W1 first (phase-1 critical path), then
                # everything pass-1/2 needs.
                ab_shard = dram.tile([NODES_CORE, 2 * F_MID], bf16)
                with (
                    tc.tile_pool(name="ph1", bufs=1) as p1,
                    tc.tile_pool(name="psum1", bufs=1, space="PSUM") as pp1,
                ):
                    # inT[:, kk, n] = x[n, kk*128 + p]; host supplies x^T
                    inT = p1.tile([128, KC_IN, NODES_CORE], bf16)
                    nc.sync.dma_start(
                        out=inT[:],
                        in_=inpT[:].rearrange("(k p) n -> p k n", p=128))
                    w1_sb = p1.tile([128, 2 * KC_IN, F_MID], bf16)  # W1 rows chunked
                    for kc in range(2 * KC_IN):
                        nc.sync.dma_start(out=w1_sb[:, kc, :],
                                          in_=w1[kc * 128:(kc + 1) * 128, :])

                    for t in range(NODES_CORE // 128):
                        for half in range(2):           # A then B
                            for ofc in range(2):        # 512-wide output chunks
                                mmps = pp1.tile([128, 512], f32, tag="mmps", bufs=3)
                                for kk in range(KC_IN):
                                    nc.tensor.matmul(
                                        out=mmps[:],
                                        lhsT=inT[:, kk, t * 128:(t + 1) * 128],
                                        rhs=w1_sb[:, half * KC_IN + kk,
                                                  ofc * 512:(ofc + 1) * 512],
                                        start=(kk == 0), stop=(kk == KC_IN - 1),
                                    )
                                absb = p1.tile([128, 512], bf16, tag="absb", bufs=8)
                                ceng = nc.vector if (t * 4 + half * 2 + ofc) % 2 else nc.scalar
                                if ceng is nc.scalar:
                                    ceng.activation(out=absb[:], in_=mmps[:],
                                                    func=AF.Identity)
                                else:
                                    ceng.tensor_copy(out=absb[:], in_=mmps[:])
                                nc.sync.dma_start(
                                    out=ab_shard[t * 128:(t + 1) * 128,
                                                 half * F_MID + ofc * 512:
                                                 half * F_MID + (ofc + 1) * 512],
                                    in_=absb[:])

                # setup loads AFTER the phase-1 writes in the DMA queue so
                # they don't head-of-line block the ab_shard writes; they
                # complete during the AllGather window.
                nc.sync.dma_start(out=idxs[:, 0, :], in_=idx_src[:])
                nc.sync.dma_start(out=idxs[:, 1, :], in_=idx_dst[:])
                nc.sync.dma_start(out=w2_sb[:],
                                  in_=w2[:].rearrange("(c p) n -> p c n", p=128))
                nc.sync.dma_start(out=gam[:],
                                  in_=gamma[:].rearrange("(c p) -> p c", p=128))
                nc.sync.dma_start(out=bet[:],
                                  in_=beta[:].rearrange("(c p) -> p c", p=128))
                nc.sync.dma_start(out=b2_sb[:], in_=b2[:, None])
                nc.gpsimd.memset(eps_t[:], BN_EPS)

                ab_full = dram.tile([N_NODES, 2 * F_MID], bf16,
                                    addr_space="Local" if for_timeline else "Shared")
                if for_timeline:
                    if "coll" not in ABLATE:
                        nc.sync.dma_start(out=ab_full[0:NODES_CORE, :], in_=ab_shard[:])
                else:
                    nc.gpsimd.collective_compute(
                        "AllGather", OP.bypass, replica_groups=groups,
                        ins=[ab_shard.opt()], outs=[ab_full.opt()])

                # ---------------- pass 1: gather + h + stats ----------------
                do_pass1 = "stop1" not in ABLATE
                do_stats = do_pass1 and "stop2" not in ABLATE
                do_pass2 = do_stats and "stop3" not in ABLATE

                h_scr = dram.tile([NT - N_CACHE, 128, FC, GE], bf16)

                # tile visit order: interleave spilled tiles among cached ones
                # so their extra spill/reload DMA hides behind cached-tile
                # compute in both passes.
                n_spill = NT - N_CACHE
                seq = []          # (tile_id, cache_slot or None, spill_slot or None)
                ci = si = 0
                for g in range(NT):
                    if si < n_spill and (g % 2 == 1 or ci >= N_CACHE):
                        seq.append((g, None, si)); si += 1
                    else:
                        seq.append((g, ci, None)); ci += 1

                N_POOL_ADD = 3    # h+=B chunks done on Pool (rest on DVE)

                with tc.tile_pool(name="hc", bufs=1) as hcp:
                    hcache = hcp.tile([128, N_CACHE, FC, GE], bf16)
                    haps = {}

                    def p1_gather(k):
                        g, cslot, _ = seq[k]
                        if cslot is not None:
                            hap = hcache[:, cslot, :, :]
                        else:
                            hh1 = sb.tile([128, FC, GE], bf16, tag="h", bufs=2)
                            hap = hh1[:]
                        haps[k] = hap
                        bgt = sb.tile([128, FC, GE], bf16, tag="bg", bufs=2)
                        isl = slice(g * (GE // 16), (g + 1) * (GE // 16))
                        if "gathers" not in ABLATE:
                            nc.gpsimd.dma_gather(
                                hap, ab_full[:, 0:F_MID], idxs[:, 0, isl],
                                GE, GE, F_MID, elem_step=2 * F_MID, transpose=True)
                            nc.gpsimd.dma_gather(
                                bgt[:], ab_full[:, F_MID:2 * F_MID],
                                idxs[:, 1, isl],
                                GE, GE, F_MID, elem_step=2 * F_MID, transpose=True)
                        return bgt

                    def p1_compute(k, bgt):
                        g, _, sslot = seq[k]
                        hap = haps[k]
                        for c in range(FC):
                            eng = nc.gpsimd if c >= FC - N_POOL_ADD else nc.vector
                            eng.tensor_tensor(out=hap[:, c, :], in0=hap[:, c, :],
                                              in1=bgt[:, c, :], op=OP.add)
                        if "bnstats" not in ABLATE:
                            last = (k == NT - 1)
                            for c in range(FC):
                                nc.vector.bn_stats(out=stats[:, c, g, :],
                                                   in_=hap[:, c, :])
                                if last:
                                    nc.vector.bn_aggr(out=mv[:, c, :],
                                                      in_=stats[:, c, :, :])
                        if sslot is not None and "spill" not in ABLATE:
                            nc.sync.dma_start(out=h_scr[sslot], in_=hap)

                    if do_pass1:
                        prev_bg = p1_gather(0)
                        for k in range(NT):
                            nxt_bg = p1_gather(k + 1) if k + 1 < NT else None
                            p1_compute(k, prev_bg)
                            prev_bg = nxt_bg

                    # ---------------- stats: aggregate + AllReduce ----------------
                    if not do_stats:
                        raise _StopBuild
                    nc.vector.reciprocal(out=rgam[:], in_=gam[:])
                    nc.vector.tensor_scalar_mul(out=ar_sb[:, 0:FC], in0=mv[:, :, 0],
                                                scalar1=float(E_CORE))
                    nc.vector.tensor_tensor(out=msq[:], in0=mv[:, :, 0],
                                            in1=mv[:, :, 0], op=OP.mult)
                    nc.vector.tensor_tensor(out=msq[:], in0=msq[:], in1=mv[:, :, 1],
                                            op=OP.add)
                    nc.vector.tensor_scalar_mul(out=ar_sb[:, FC:2 * FC], in0=msq[:],
                                                scalar1=float(E_CORE))
                    ar_in = dram.tile([128, 2 * FC], f32)
                    ar_out = dram.tile([128, 2 * FC], f32,
                                       addr_space="Local" if for_timeline else "Shared")
                    nc.sync.dma_start(out=ar_in[:], in_=ar_sb[:])
                    if for_timeline:
                        if "coll" not in ABLATE:
                            nc.sync.dma_start(out=ar_out[:], in_=ar_in[:])
                    else:
                        nc.gpsimd.collective_compute(
                            "AllReduce", OP.add, replica_groups=groups,
                            ins=[ar_in.opt()], outs=[ar_out.opt()])
                    if for_timeline and "coll" in ABLATE:
                        nc.sync.dma_start(out=gsum[:], in_=ar_in[:])
                    else:
                        nc.sync.dma_start(out=gsum[:], in_=ar_out[:])

                    # mu = gsum[0:FC]/2E, E[h^2] = gsum[FC:]/2E (one op)
                    inv_n = 1.0 / (2.0 * E)
                    nc.vector.tensor_scalar_mul(out=musig[:], in0=gsum[:],
                                                scalar1=inv_n)
                    mu = musig[:, 0:FC]
                    var = musig[:, FC:2 * FC]
                    nc.vector.tensor_tensor(out=musq[:], in0=mu, in1=mu,
                                            op=OP.mult)
                    nc.vector.tensor_tensor(out=var, in0=var, in1=musq[:],
                                            op=OP.subtract)
                    nc.scalar.activation(out=std[:], in_=var, func=AF.Sqrt,
                                         bias=eps_t[:, 0:1])
                    nc.vector.reciprocal(out=rstd[:], in_=std[:])

                    # refold (scale = gamma*rstd > 0 since gamma > 0):
                    #   W2' = scale ⊙rows W2;  cb = shift/scale = beta/scale - mu
                    nc.vector.tensor_tensor(out=scale[:], in0=gam[:], in1=rstd[:],
                                            op=OP.mult)
                    nc.vector.tensor_tensor(out=inv_s[:], in0=std[:], in1=rgam[:],
                                            op=OP.mult)
                    nc.vector.tensor_tensor(out=cb[:], in0=bet[:], in1=inv_s[:],
                                            op=OP.mult)
                    nc.vector.tensor_tensor(out=cb[:], in0=cb[:], in1=mu,
                                            op=OP.subtract)
                    for n in range(NCLS):
                        nc.vector.tensor_tensor(out=w2p[:, :, n], in0=w2_sb[:, :, n],
                                                in1=scale[:], op=OP.mult)

                    # ---------------- pass 2: relu(h+cb) @ W2' ----------------
                    if not do_pass2:
                        raise _StopBuild
                    def emit_out(g, ops):
                        # psum -> sbuf (+b2) on ACT, deferred 2 tiles so the
                        # in-order ACT queue never waits on PE completion.
                        ob = sb.tile([NCLS, GE], f32, tag="ob", bufs=4)
                        nc.scalar.activation(out=ob[:], in_=ops[:],
                                             func=AF.Identity,
                                             bias=b2_sb[:, 0:1], scale=1.0)
                        nc.sync.dma_start(out=outT[:, g * GE:(g + 1) * GE],
                                          in_=ob[:])

                    spilled_ks = [k for k, (_, _, s) in enumerate(seq)
                                  if s is not None]
                    hh_bufs = {}

                    def p2_reload(k):
                        _, _, sslot = seq[k]
                        hh = sb.tile([128, FC, GE], bf16, tag="h", bufs=2)
                        nc.sync.dma_start(out=hh[:], in_=h_scr[sslot])
                        hh_bufs[k] = hh

                    for j in range(min(2, len(spilled_ks))):
                        p2_reload(spilled_ks[j])
                    next_rl = 2

                    def p2_relu(k):
                        # max(h + cb, 0) in place: 6 chunks on DVE (2x mode),
                        # 2 on Pool. Runs RELU_AHEAD tiles ahead of the
                        # matmuls so PE waits are pre-satisfied and it never
                        # drops out of full clock.
                        nonlocal next_rl
                        g, cslot, sslot = seq[k]
                        if cslot is not None:
                            hhap = hcache[:, cslot, :, :]
                        else:
                            hhap = hh_bufs.pop(k)[:]
                            if next_rl < len(spilled_ks):
                                p2_reload(spilled_ks[next_rl])
                                next_rl += 1
                        for c in range(FC):
                            if c == FC - 1:
                                nc.scalar.activation(out=hhap[:, c, :],
                                                     in_=hhap[:, c, :],
                                                     func=AF.Relu,
                                                     bias=cb[:, c:c + 1],
                                                     scale=1.0)
                                continue
                            eng = nc.gpsimd if c == FC - 2 else nc.vector
                            eng.tensor_scalar(
                                out=hhap[:, c, :], in0=hhap[:, c, :],
                                scalar1=cb[:, c:c + 1], scalar2=0.0,
                                op0=OP.add, op1=OP.max)
                        return hhap

                    RELU_AHEAD = 2
                    relu_done = {}
                    for k in range(min(RELU_AHEAD, NT)):
                        relu_done[k] = p2_relu(k)
                    pending = []
                    for k in range(NT):
                        if k + RELU_AHEAD < NT:
                            relu_done[k + RELU_AHEAD] = p2_relu(k + RELU_AHEAD)
                        g = seq[k][0]
                        hhap = relu_done.pop(k)
                        ops = pp.tile([NCLS, GE], f32, tag="ops", bufs=5)
                        for c in range(FC):
                            nc.tensor.matmul(out=ops[:], lhsT=w2p[:, c, :],
                                             rhs=hhap[:, c, :],
                                             start=(c == 0), stop=(c == FC - 1))
                        pending.append((g, ops))
                        if len(pending) > 2:
                            emit_out(*pending.pop(0))
                    for gg, oo in pending:
                        emit_out(gg, oo)

            except _StopBuild:
                pass
    nc.compile()
    return nc


_NC = None


def _get_program():
    global _NC
    if _NC is None:
        _NC = build_program()
    return _NC


def _wrap_idx(col):
    """[E_CORE] int -> [128, E_CORE//16] int16 in dma_gather's wrapped layout."""
    w = col.astype(np.int16).reshape(-1, 16).T          # [16, E_CORE//16]
    return np.ascontiguousarray(np.tile(w, (8, 1)))     # replicate to 128 parts


def _to_bf16_bytes(a):
    """f32 ndarray -> bf16 (round-to-nearest-even)."""
    import ml_dtypes
    return np.asarray(a, dtype=np.float32).astype(ml_dtypes.bfloat16)


def _to_fp8(a):
    """f32 ndarray -> float8_e4m3 (TRN2 fp8e4)."""
    import ml_dtypes
    return np.asarray(a, dtype=np.float32).astype(ml_dtypes.float8_e4m3)


def make_in_maps(input, conn_idx, disconn_idx, W1, gamma, beta, W2, b2):
    input = np.asarray(input, dtype=np.float32)
    W1 = np.asarray(W1, dtype=np.float32)
    W2 = np.ascontiguousarray(np.asarray(W2, dtype=np.float32))
    gamma = np.ascontiguousarray(np.asarray(gamma, dtype=np.float32))
    beta = np.ascontiguousarray(np.asarray(beta, dtype=np.float32))
    b2 = np.ascontiguousarray(np.asarray(b2, dtype=np.float32))
    conn_idx = np.asarray(conn_idx)
    disconn_idx = np.asarray(disconn_idx)

    w1_bf = _to_bf16_bytes(W1)
    inT_bf = _to_bf16_bytes(input.T)                    # [F_IN, N]

    in_maps = []
    ec2 = E_CORE // 2  # edges per core from each of conn/disconn
    for c in range(NCORES):
        pc = np.concatenate(
            [conn_idx[c * ec2:(c + 1) * ec2], disconn_idx[c * ec2:(c + 1) * ec2]],
            axis=0)  # [E_CORE, 2]
        in_maps.append({
            "inpT_shard": np.ascontiguousarray(
                inT_bf[:, c * NODES_CORE:(c + 1) * NODES_CORE]),
            "w1": w1_bf, "w2": W2, "gamma": gamma, "beta": beta, "b2": b2,
            "idx_src": _wrap_idx(pc[:, 0]),
            "idx_dst": _wrap_idx(pc[:, 1]),
        })
    return in_maps


def assemble_output(results):
    out = np.empty((2 * E, NCLS), dtype=np.float32)
    ec2 = E_CORE // 2
    for c in range(NCORES):
        r = results[c]["outT"]  # [NCLS, E_CORE]
        out[c * ec2:(c + 1) * ec2] = r[:, 0:ec2].T
        out[E + c * ec2:E + (c + 1) * ec2] = r[:, ec2:].T
    return out


def run(inputs, trace=False):
    nc = _get_program()
    in_maps = make_in_maps(
        inputs["input"], inputs["conn_idx"], inputs["disconn_idx"],
        inputs["W1"], inputs["gamma"], inputs["beta"], inputs["W2"],
        inputs["b2"])
    res = run_bass_kernel_spmd(nc, in_maps, list(range(NCORES)), trace=trace)
    return assemble_output(res.results), res


def kernel(**inputs):
    out, _ = run(inputs, trace=False)
    return out
